# revision 6
# baseline (speedup 1.0000x reference)
"""CNN+GAT kernel for Trainium2, 8 NeuronCores, data-parallel over the batch.

Problem (hardcoded): B=16 graphs, L=384 timesteps, N=128 nodes, E=4096 edges.
Per graph: 4-layer 1D CNN (1->32->64->128->256, k=3 SAME, ReLU) over each
node's series, mean-pool over time, FC 256->256, then 3x (GATConv + GraphNorm
+ residual ReLU), mean-pool over nodes, linear classifier -> scalar.

Sharding: 2 graphs per core.  The wall-clock of one call is dominated by
host->device transfer over the axon tunnel (~12 ms/MB) plus ~190 ms fixed
dispatch cost, so the input payload is minimized:
 - x ships as [2, 4, 12352] bf16 per core (198 KB): chunk-major flattened
   series with a zero-padded timestep on each side; the 3 conv taps are
   built on-device as 32-col-shifted DMAs of the same rows.
 - all weights ship in ONE compact bf16 pack; each core receives a 1/8
   partition-shard (111 KB) and the full pack is reassembled on-device with
   an AllGather, then expanded (chunk duplication for conv2/conv3) into SBUF.
 - per-partition conv biases ship as [128, 8] f32; all row-vector params
   ([1,256] FC/GAT/norm/cls rows) ship as one [1, 3600] f32 row and the
   [128,256] broadcasts are materialized on-device with K=1 ones-matmuls.

Compute structure (unchanged from the tuned baseline):
 - Conv layers run on the TensorEngine in bf16 with nodes interleaved along
   the free dim; taps accumulated in PSUM; ReLU+bias on ScalarE/VectorE
   while copying PSUM->SBUF; strided VectorE tensor_reduce time pooling
   (1/384 folded into the FC weight host-side).
 - Dense GAT edge softmax against the host-built edge multiplicity matrix;
   messages/denominators as matmuls; GraphNorm via ones-vector matmuls.
"""
import numpy as np
import ml_dtypes


def _enable_jax_compile_cache():
    # run_bass_kernel_spmd re-jits a fresh closure per call, so XLA's
    # in-memory executable cache never hits and every call re-runs the
    # (walrus) BIR compile (~0.5 s).  The persistent compilation cache is
    # checked before backend_compile and turns that into a disk hit.
    try:
        import jax
        if jax.config.jax_compilation_cache_dir is None:
            jax.config.update("jax_compilation_cache_dir",
                              "/tmp/jax_exec_cache")
        jax.config.update("jax_persistent_cache_min_compile_time_secs", 0)
        jax.config.update("jax_persistent_cache_min_entry_size_bytes", 0)
    except Exception:
        pass


B, L, N, E = 16, 384, 128, 4096
H, C, F = 4, 64, 256
EPS = 1e-5
GC = 32                 # nodes per conv chunk
TC = (L + 2) * GC       # padded cols per chunk = 12352
LGC = L * GC            # 12288
NSLICE = L * GC // 512  # 512-col psum slices per chunk = 24
GPC = 2                 # graphs per core
NCORES = 8
ALLGATHER = True        # ship 1/8 weight shard per core, AllGather on device

_BF16 = ml_dtypes.bfloat16
_cache = {}

# ---- DRAM weight pack (bf16 [128, WC]) region offsets (compact) ----
P_WC1 = 0      # [128, 32]
P_WC2 = 32     # [96, 64]   k at rows 32k
P_WC3A = 96    # [128, 128] k0 rows 0-63, k1 rows 64-127
P_WC3B = 224   # [64, 128]  k2 rows 0-63
P_WC4 = 352    # [128, 768] k,m major: 128*(2k+m)
P_FCW = 1120   # [128, 512] fcwT t=0,1
P_WTT = 1632   # [128, 1536] wtT l,t major: 256*(2l+t)
P_WAS = 3168   # [128, 48]  wasad l,t major: 8*(2l+t)
P_CNT = 3216   # [128, 128]
P_ID = 3344    # [128, 128]
WC = 3472

# ---- SBUF weight tile (bf16 [128, WSB]) offsets (expanded) ----
S_WC1 = 0
S_WC2 = 32     # +64k, all 4 row-groups duplicated
S_WC3 = 224    # +128k, both 64-row halves duplicated
S_WC4 = 608    # +128*(2k+m)
S_FCW = 1376   # +256t
S_WTT = 1888   # +256*(2l+t)
S_WAS = 3424   # +8*(2l+t)
S_CNT = 3472
S_ID = 3600
WSB = 3728

# ---- f32 row pack [1, RC] offsets ----
R_FCB = 0
R_GATB = 256   # +256l
R_NB = 1024    # +256l
R_MS = 1792    # +256l
R_G = 2560     # +256l
R_CLSW = 3328
R_CLSB = 3584
RC = 3600


def _build_program():
    import concourse.bacc as bacc
    import concourse.mybir as mybir
    import concourse.tile as tile

    F32 = mybir.dt.float32
    BF16 = mybir.dt.bfloat16
    AF = mybir.ActivationFunctionType
    ALU = mybir.AluOpType

    nc = bacc.Bacc("TRN2", target_bir_lowering=False, debug=False,
                   num_devices=NCORES)
    d = {}

    def par(name, shape, dt):
        d[name] = nc.dram_tensor(name, list(shape), dt, kind="ExternalInput")

    par("xprep", [GPC, 4, TC], BF16)
    if ALLGATHER:
        par("wsh", [128 // NCORES, WC], BF16)
    else:
        par("wpack", [128, WC], BF16)
    par("packc", [128, 8], F32)
    par("packr", [1, RC], F32)
    out_d = nc.dram_tensor("out", [1, GPC], F32, kind="ExternalOutput")

    with tile.TileContext(nc) as tc:
        with tc.tile_pool(name="const", bufs=1) as cp, \
             tc.tile_pool(name="dram", bufs=1, space="DRAM") as dp:
            # ---- weight source in DRAM (AllGather the 1/8 shards) ----
            if ALLGATHER:
                inb = dp.tile([128 // NCORES, WC], BF16, tag="inb")
                wsrc = dp.tile([128, WC], BF16, tag="gath")
                nc.gpsimd.dma_start(inb[:], d["wsh"][:])
                nc.gpsimd.collective_compute(
                    "AllGather", ALU.bypass,
                    replica_groups=[list(range(NCORES))],
                    ins=[inb.opt()], outs=[wsrc.opt()])
            else:
                wsrc = d["wpack"]

            dma = nc.sync.dma_start
            wts = cp.tile([128, WSB], BF16, tag="wts", name="wts")
            dma(wts[:, S_WC1:S_WC1 + 32], wsrc[:, P_WC1:P_WC1 + 32])
            for j in range(4):              # conv2: duplicate to 4 row groups
                for k in range(3):
                    dma(wts[32 * j:32 * j + 32,
                            S_WC2 + 64 * k:S_WC2 + 64 * k + 64],
                        wsrc[32 * k:32 * k + 32, P_WC2:P_WC2 + 64])
            for k in range(3):              # conv3: duplicate to both halves
                sr, scl = (64 * k, P_WC3A) if k < 2 else (0, P_WC3B)
                for h in range(2):
                    dma(wts[64 * h:64 * h + 64,
                            S_WC3 + 128 * k:S_WC3 + 128 * k + 128],
                        wsrc[sr:sr + 64, scl:scl + 128])
            dma(wts[:, S_WC4:S_WC4 + 768], wsrc[:, P_WC4:P_WC4 + 768])
            dma(wts[:, S_FCW:S_FCW + 512], wsrc[:, P_FCW:P_FCW + 512])
            dma(wts[:, S_WTT:S_WTT + 1536], wsrc[:, P_WTT:P_WTT + 1536])
            dma(wts[:, S_WAS:S_WAS + 48], wsrc[:, P_WAS:P_WAS + 48])
            dma(wts[:, S_CNT:S_CNT + 128], wsrc[:, P_CNT:P_CNT + 128])
            dma(wts[:, S_ID:S_ID + 128], wsrc[:, P_ID:P_ID + 128])

            pct = cp.tile([128, 8], F32, tag="pct")
            dma(pct[:], d["packc"][:])
            rowt = cp.tile([1, RC], F32, tag="rowt")
            dma(rowt[:], d["packr"][:])

            ones_row_f = cp.tile([1, 128], F32, tag="onr")
            nc.vector.memset(ones_row_f[:], 1.0)
            ones_col = cp.tile([128, 1], F32, tag="onc")
            nc.vector.memset(ones_col[:], 1.0 / N)
            ones_row_bf = cp.tile([1, 128], BF16, tag="onrb")
            nc.vector.memset(ones_row_bf[:], 1.0)
            ones_col_bf = cp.tile([128, 1], BF16, tag="oncb")
            nc.vector.memset(ones_col_bf[:], 1.0)

            # materialize the [128,256] row broadcasts: fcb, gatb*3, nb*3
            bct = cp.tile([128, 7 * 256], F32, tag="bct")
            bc_offs = [R_FCB] + [R_GATB + 256 * l for l in range(3)] \
                + [R_NB + 256 * l for l in range(3)]
            with tc.tile_pool(name="bcps", bufs=2, space="PSUM") as pb:
                for i, off in enumerate(bc_offs):
                    ps = pb.tile([128, 256], F32, tag="bc")
                    nc.tensor.matmul(ps[:], ones_row_f[:],
                                     rowt[0:1, off:off + 256],
                                     start=True, stop=True)
                    nc.vector.tensor_copy(bct[:, 256 * i:256 * i + 256], ps[:])

            cntf = cp.tile([128, 128], F32, tag="cntf")
            nc.vector.tensor_copy(cntf[:], wts[:, S_CNT:S_CNT + 128])

            ct = {}
            ct["wc1"] = wts[:, S_WC1:S_WC1 + 32]
            for k in range(3):
                ct[f"wc2k{k}"] = wts[:, S_WC2 + 64 * k:S_WC2 + 64 * k + 64]
                ct[f"wc3k{k}"] = wts[:, S_WC3 + 128 * k:S_WC3 + 128 * k + 128]
                for m in range(2):
                    o = S_WC4 + 128 * (2 * k + m)
                    ct[f"wc4k{k}m{m}"] = wts[:, o:o + 128]
            for t in range(2):
                ct[f"fcwT{t}"] = wts[:, S_FCW + 256 * t:S_FCW + 256 * t + 256]
            for l in range(3):
                for t in range(2):
                    o = S_WTT + 256 * (2 * l + t)
                    ct[f"wtT{l}t{t}"] = wts[:, o:o + 256]
                    o = S_WAS + 8 * (2 * l + t)
                    ct[f"wasad{l}t{t}"] = wts[:, o:o + 8]
                ct[f"gatb_bc{l}"] = bct[:, 256 * (1 + l):256 * (1 + l) + 256]
                ct[f"nb_bc{l}"] = bct[:, 256 * (4 + l):256 * (4 + l) + 256]
                o = R_MS + 256 * l
                ct[f"msrow{l}"] = rowt[0:1, o:o + 256]
                o = R_G + 256 * l
                ct[f"grow{l}"] = rowt[0:1, o:o + 256]
            ct["fcb_bc"] = bct[:, 0:256]
            ct["cntT"] = cntf[:]
            ct["ident"] = wts[:, S_ID:S_ID + 128]
            for i, nm in enumerate(("bias1", "bias2", "bias3", "bias4a",
                                    "bias4b")):
                ct[nm] = pct[:, i:i + 1]
            ct["clsw"] = rowt[0:1, R_CLSW:R_CLSW + 256]
            ct["clsb"] = rowt[0:1, R_CLSB:R_CLSB + 1]
            ct["ones_col"] = ones_col[:]
            ct["ones_row_f"] = ones_row_f[:]
            ct["ones_row_bf"] = ones_row_bf[:]
            ct["ones_col_bf"] = ones_col_bf[:]

            out_sb = cp.tile([1, GPC], F32, tag="out_sb")
            dots = cp.tile([1, GPC], F32, tag="dots")

            poolfs = [[cp.tile([128, 128], BF16, tag=f"pool{g}_{m}",
                                name=f"poolf{g}_{m}") for m in range(2)]
                      for g in range(GPC)]
            # ---------------- conv1..conv4, both graphs ----------------
            # one shared SBUF pool; slot "A" rotates x(g0)->c2(g0)->x(g1)->
            # c2(g1), slot "B" rotates c1(g0)->c3(g0)->c1(g1)->c3(g1); the
            # next graph's x DMA is issued before conv4 (slot A is idle then)
            with tc.tile_pool(name="conv", bufs=1) as pc:

                def alloc_xt(g):
                    # tap row 32j+k is the chunk-j series shifted by 32(k-1)
                    # cols; the shipped row has 32 zero cols at each end so
                    # all three shifts read in-bounds.
                    t = pc.tile([128, TC], BF16, tag="A", name=f"xt{g}",
                                padded_shape=[128, 2 * TC])
                    for j in range(4):
                        for k in range(3):
                            nc.sync.dma_start(
                                t[32 * j + k:32 * j + k + 1, GC:GC + LGC],
                                d["xprep"][g][j:j + 1, 32 * k:32 * k + LGC])
                    return t
                xts = [None] * GPC
                xts[0] = alloc_xt(0)
                for g in range(GPC):
                    if True:
                        poolf = poolfs[g]
                        xt = xts[g]
                        c1 = pc.tile([128, TC], BF16, tag="B", name="c1",
                                     padded_shape=[128, 4 * TC])
                        nc.vector.memset(c1[:, 0:GC], 0.0)
                        nc.vector.memset(c1[:, TC - GC:TC], 0.0)
                        # conv1: K=3 (taps stacked), 4 chunks on diagonal
                        # tiles; 4 slices per psum tile, one ACT per group
                        ps1cm = tc.tile_pool(name=f"g{g}ps1", bufs=2, space="PSUM")
                        ps1 = ps1cm.__enter__()
                        for sg in range(NSLICE // 4):
                            lo = GC + 2048 * sg
                            pt = ps1.tile([128, 2048], F32, tag="cps")
                            for si in range(4):
                                for j in range(4):
                                    nc.tensor.matmul(
                                        pt[32 * j:32 * j + 32, 512 * si:512 * si + 512],
                                        ct["wc1"][32 * j:32 * j + 3, :],
                                        xt[32 * j:32 * j + 3,
                                           lo + 512 * si:lo + 512 * si + 512],
                                        start=True, stop=True,
                                        tile_position=(32 * j, 32 * j))
                            nc.vector.tensor_scalar(
                                c1[:, lo:lo + 2048], pt[:], ct["bias1"],
                                0.0, op0=ALU.add, op1=ALU.max)
                        c2 = pc.tile([128, 2 * TC], BF16, tag="A", name="c2")
                        for b in range(2):
                            nc.vector.memset(c2[:, b * TC:b * TC + GC], 0.0)
                            nc.vector.memset(c2[:, (b + 1) * TC - GC:(b + 1) * TC], 0.0)
                        ps1cm.__exit__(None, None, None)
                        ps2cm = tc.tile_pool(name=f"g{g}ps2", bufs=4, space="PSUM")
                        ps2 = ps2cm.__enter__()
                        # conv2: per-tap K=32, 4 chunks concurrent; 2 slices
                        # per psum tile pair, one ACT per (group, block)
                        for sg in range(NSLICE // 2):
                            lo = GC + 1024 * sg
                            pts = [ps2.tile([128, 1024], F32, tag="hps", name=f"c2ps{i}")
                                   for i in range(2)]
                            for si in range(2):
                                s = 2 * sg + si
                                for j in range(4):
                                    pt = pts[j // 2]
                                    ro = 64 * (j % 2)
                                    for k in range(3):
                                        nc.tensor.matmul(
                                            pt[ro:ro + 64, 512 * si:512 * si + 512],
                                            ct[f"wc2k{k}"][32 * j:32 * j + 32, :],
                                            c1[32 * j:32 * j + 32,
                                               512 * s + GC * k:512 * s + GC * k + 512],
                                            start=(k == 0), stop=(k == 2),
                                            tile_position=(32 * j, ro))
                            for b in range(2):
                                nc.scalar.activation(
                                    c2[:, b * TC + lo:b * TC + lo + 1024], pts[b][:],
                                    AF.Relu, bias=ct["bias2"][:])
                        c3 = pc.tile([128, 4 * TC], BF16, tag="B", name="c3")
                        for b in range(4):
                            nc.vector.memset(c3[:, b * TC:b * TC + GC], 0.0)
                            nc.vector.memset(c3[:, (b + 1) * TC - GC:(b + 1) * TC], 0.0)
                        # conv3: per-tap K=64; chunk j reads c2 rows 64*(j%2),
                        # col-block j//2; writes c3 col-block j (full 128 rows)
                        for blk in range(2):
                            for sg in range(NSLICE // 2):
                                lo = GC + 1024 * sg
                                pts = [ps2.tile([128, 1024], F32, tag="hps", name=f"c3ps{i}")
                                       for i in range(2)]
                                for half in range(2):
                                    j = 2 * blk + half
                                    ro = 64 * half
                                    for si in range(2):
                                        s = 2 * sg + si
                                        for k in range(3):
                                            nc.tensor.matmul(
                                                pts[half][:, 512 * si:512 * si + 512],
                                                ct[f"wc3k{k}"][ro:ro + 64, :],
                                                c2[ro:ro + 64,
                                                   blk * TC + 512 * s + GC * k:
                                                   blk * TC + 512 * s + GC * k + 512],
                                                start=(k == 0), stop=(k == 2),
                                                tile_position=(ro, 0))
                                    nc.scalar.activation(
                                        c3[:, j * TC + lo:j * TC + lo + 1024],
                                        pts[half][:], AF.Relu, bias=ct["bias3"][:])
                        ps2cm.__exit__(None, None, None)
                        if g + 1 < GPC:
                            xts[g + 1] = alloc_xt(g + 1)
                        ps3cm = tc.tile_pool(name=f"g{g}ps3", bufs=2, space="PSUM")
                        ps3 = ps3cm.__enter__()
                        # -------- conv4 + groupwise time pool --------
                        for j in range(4):
                            for m in range(2):
                                partials = pc.tile([128, 192], F32, tag="pp",
                                                   bufs=2, name="partials")
                                for sg in range(NSLICE // 4):
                                    pt = ps3.tile([128, 2048], F32, tag="cps",
                                                  name="c4pt")
                                    for si in range(4):
                                        s = 4 * sg + si
                                        for k in range(3):
                                            nc.tensor.matmul(
                                                pt[:, 512 * si:512 * si + 512],
                                                ct[f"wc4k{k}m{m}"][:],
                                                c3[:, j * TC + 512 * s + GC * k:
                                                      j * TC + 512 * s + GC * k + 512],
                                                start=(k == 0), stop=(k == 2))
                                    c4sl = pc.tile([128, 2048], BF16, tag="c4sl",
                                                   bufs=4, name="c4sl")
                                    nc.scalar.activation(
                                        c4sl[:], pt[:], AF.Relu,
                                        bias=ct["bias4a" if m == 0 else "bias4b"][:])
                                    nc.vector.tensor_reduce(
                                        partials[:, 32 * sg:32 * sg + 32],
                                        c4sl[:].rearrange("p (t n) -> p n t", n=GC),
                                        axis=mybir.AxisListType.X, op=ALU.add)
                                with nc.allow_low_precision(
                                        reason="bf16 pooled features feed a "
                                               "bf16 FC matmul; DVE reduce "
                                               "accumulates fp32 internally"):
                                    nc.vector.tensor_reduce(
                                        poolf[m][:, GC * j:GC * j + GC],
                                        partials[:].rearrange("p (s n) -> p n s", n=GC),
                                        axis=mybir.AxisListType.X, op=ALU.add)
                        ps3cm.__exit__(None, None, None)
            # ---------- FC + GAT, both graphs interleaved ----------
            with tc.tile_pool(name="gat", bufs=2) as gp, \
                 tc.tile_pool(name="gatx", bufs=4) as gx, \
                 tc.tile_pool(name="psC", bufs=2, space="PSUM") as psc:
                Xs = [None] * GPC
                for g in range(GPC):
                    poolf = poolfs[g]
                    fc_ps = psc.tile([128, 256], F32, tag="T2")
                    for m in range(2):
                        nc.tensor.matmul(fc_ps[:], poolf[m][:],
                                         ct[f"fcwT{m}"][:],
                                         start=(m == 0), stop=(m == 1))
                    X = gx.tile([128, 256], F32, tag="X")
                    nc.vector.tensor_tensor(X[:], fc_ps[:], ct["fcb_bc"][:],
                                            op=ALU.add)
                    Xs[g] = X
                for l in range(3):
                    for g in range(GPC):
                        X = Xs[g]
                        X_bf = gp.tile([128, 256], BF16, tag="xbf")
                        nc.vector.tensor_copy(X_bf[:], X[:])
                        xfm_ps = psc.tile([128, 256], BF16, tag="T1")
                        for t in range(2):
                            nc.tensor.transpose(
                                xfm_ps[:, 128 * t:128 * t + 128],
                                X_bf[:, 128 * t:128 * t + 128], ct["ident"][:])
                        xfm_bf = gp.tile([128, 256], BF16, tag="xfm")
                        nc.vector.tensor_copy(xfm_bf[:], xfm_ps[:])

                        h_ps = psc.tile([128, 256], F32, tag="T2")
                        alnm_ps = psc.tile([128, 8], F32, tag="T3")
                        aldf_ps = psc.tile([1, 512], F32, tag="T4")
                        for t in range(2):
                            nc.tensor.matmul(h_ps[:],
                                             xfm_bf[:, 128 * t:128 * t + 128],
                                             ct[f"wtT{l}t{t}"][:],
                                             start=(t == 0), stop=(t == 1))
                            nc.tensor.matmul(alnm_ps[:],
                                             xfm_bf[:, 128 * t:128 * t + 128],
                                             ct[f"wasad{l}t{t}"][:],
                                             start=(t == 0), stop=(t == 1))
                            for hh in range(4):
                                nc.tensor.matmul(
                                    aldf_ps[0:1, 128 * hh:128 * hh + 128],
                                    ct[f"wasad{l}t{t}"][:, 4 + hh:5 + hh],
                                    xfm_bf[:, 128 * t:128 * t + 128],
                                    start=(t == 0), stop=(t == 1))
                        hnm_bf = gp.tile([128, 256], BF16, tag="hnm")
                        nc.vector.tensor_copy(hnm_bf[:], h_ps[:])
                        alnm = gp.tile([128, 8], F32, tag="alnm")
                        nc.vector.tensor_copy(alnm[:], alnm_ps[:])
                        aldf = gp.tile([1, 512], BF16, tag="aldf")
                        nc.vector.tensor_copy(aldf[:], aldf_ps[:])

                        lg_ps = psc.tile([128, 512], F32, tag="T1")
                        for hh in range(4):
                            nc.tensor.matmul(
                                lg_ps[:, 128 * hh:128 * hh + 128],
                                ct["ones_row_bf"][:],
                                aldf[0:1, 128 * hh:128 * hh + 128],
                                start=True, stop=True)
                        # leaky(lg + al_s) on DVE (avoids Prelu ACT
                        # table churn), then exp on ACT
                        lr = gp.tile([128, 512], F32, tag="lr")
                        for hh in range(4):
                            nc.vector.tensor_scalar_add(
                                lr[:, 128 * hh:128 * hh + 128],
                                lg_ps[:, 128 * hh:128 * hh + 128],
                                alnm[:, hh:hh + 1])
                        lr2 = gp.tile([128, 512], F32, tag="lr2")
                        nc.vector.scalar_tensor_tensor(
                            lr2[:], lr[:], 0.2, lr[:],
                            op0=ALU.mult, op1=ALU.max)
                        ex = gp.tile([128, 512], F32, tag="ex")
                        nc.scalar.activation(ex[:], lr2[:], AF.Exp)
                        exT = gp.tile([128, 512], BF16, tag="exT")
                        cnt_bc = ct["cntT"].rearrange(
                            "p (h i) -> p h i", h=1).broadcast_to([128, 4, 128])
                        nc.vector.tensor_tensor(
                            exT[:].rearrange("p (h i) -> p h i", h=4),
                            ex[:].rearrange("p (h i) -> p h i", h=4),
                            cnt_bc, op=ALU.mult)

                        msg_ps = psc.tile([128, 256], F32, tag="T2")
                        s_ps = psc.tile([128, 4], F32, tag="T3")
                        for hh in range(4):
                            nc.tensor.matmul(
                                msg_ps[:, 64 * hh:64 * hh + 64],
                                exT[:, 128 * hh:128 * hh + 128],
                                hnm_bf[:, 64 * hh:64 * hh + 64],
                                start=True, stop=True)
                            nc.tensor.matmul(
                                s_ps[:, hh:hh + 1],
                                exT[:, 128 * hh:128 * hh + 128],
                                ct["ones_col_bf"][:],
                                start=True, stop=True)
                        r2 = gp.tile([128, 4], F32, tag="r2")
                        nc.vector.reciprocal(r2[:], s_ps[:])
                        y = gp.tile([128, 256], F32, tag="y")
                        for hh in range(4):
                            nc.vector.scalar_tensor_tensor(
                                y[:, 64 * hh:64 * hh + 64],
                                msg_ps[:, 64 * hh:64 * hh + 64],
                                r2[:, hh:hh + 1],
                                ct[f"gatb_bc{l}"][:, 64 * hh:64 * hh + 64],
                                op0=ALU.mult, op1=ALU.add)
                        # GraphNorm
                        mu_ps = psc.tile([1, 256], F32, tag="T4")
                        nc.tensor.matmul(mu_ps[:], ct["ones_col"][:], y[:],
                                         start=True, stop=True)
                        msmu = gp.tile([1, 256], F32, tag="msmu")
                        nc.vector.tensor_tensor(msmu[:], mu_ps[:],
                                                ct[f"msrow{l}"][:], op=ALU.mult)
                        msmub_ps = psc.tile([128, 256], F32, tag="T4")
                        nc.tensor.matmul(msmub_ps[:], ct["ones_row_f"][:],
                                         msmu[:], start=True, stop=True)
                        o = gp.tile([128, 256], F32, tag="o")
                        nc.vector.tensor_tensor(o[:], y[:], msmub_ps[:],
                                                op=ALU.subtract)
                        sq = gp.tile([128, 256], F32, tag="sq")
                        nc.vector.tensor_tensor(sq[:], o[:], o[:], op=ALU.mult)
                        var_ps = psc.tile([1, 256], F32, tag="T1")
                        nc.tensor.matmul(var_ps[:], ct["ones_col"][:], sq[:],
                                         start=True, stop=True)
                        # rstd = 1/sqrt(var+eps) via bit-trick + Newton
                        # iteration on DVE (avoids ACT table churn)
                        ve = gp.tile([1, 256], F32, tag="ve")
                        nc.vector.tensor_scalar_add(ve[:], var_ps[:], EPS)
                        magic = gp.tile([1, 256], mybir.dt.int32, tag="magic")
                        nc.vector.memset(magic[:], 0x5F3759DF)
                        yb = gp.tile([1, 256], mybir.dt.int32, tag="yb")
                        nc.vector.tensor_scalar(
                            yb[:], ve[:].bitcast(mybir.dt.int32), 1, None,
                            op0=ALU.arith_shift_right)
                        rstd = gp.tile([1, 256], F32, tag="rstd")
                        nc.vector.tensor_tensor(
                            rstd[:].bitcast(mybir.dt.int32), magic[:], yb[:],
                            op=ALU.subtract)
                        t1r = gp.tile([1, 256], F32, tag="t1r")
                        t2r = gp.tile([1, 256], F32, tag="t2r")
                        for _ in range(1):
                            nc.vector.tensor_tensor(t1r[:], ve[:], rstd[:],
                                                    op=ALU.mult)
                            nc.vector.tensor_tensor(t2r[:], t1r[:], rstd[:],
                                                    op=ALU.mult)
                            nc.vector.tensor_scalar(t2r[:], t2r[:], -0.5, 1.5,
                                                    op0=ALU.mult, op1=ALU.add)
                            nc.vector.tensor_tensor(rstd[:], rstd[:], t2r[:],
                                                    op=ALU.mult)
                        gs = gp.tile([1, 256], F32, tag="gs")
                        nc.vector.tensor_tensor(gs[:], rstd[:],
                                                ct[f"grow{l}"][:], op=ALU.mult)
                        gsb_ps = psc.tile([128, 256], F32, tag="T2")
                        nc.tensor.matmul(gsb_ps[:], ct["ones_row_f"][:],
                                         gs[:], start=True, stop=True)
                        t1 = gp.tile([128, 256], F32, tag="t1")
                        nc.vector.tensor_tensor(t1[:], o[:], gsb_ps[:],
                                                op=ALU.mult)
                        t2 = gp.tile([128, 256], F32, tag="t2")
                        nc.vector.tensor_tensor(t2[:], t1[:], X[:], op=ALU.add)
                        t3 = gp.tile([128, 256], F32, tag="t3")
                        nc.vector.tensor_tensor(t3[:], t2[:],
                                                ct[f"nb_bc{l}"][:], op=ALU.add)
                        X = gx.tile([128, 256], F32, tag="X")
                        nc.vector.tensor_scalar_max(X[:], t3[:], 0.0)
                        Xs[g] = X
                for g in range(GPC):
                    X = Xs[g]
                    pooled_ps = psc.tile([1, 256], F32, tag="T3")
                    nc.tensor.matmul(pooled_ps[:], ct["ones_col"][:], X[:],
                                     start=True, stop=True)
                    scr = gp.tile([1, 256], F32, tag="scr")
                    nc.vector.scalar_tensor_tensor(
                        scr[:], pooled_ps[:], 1.0, ct["clsw"][:],
                        op0=ALU.mult, op1=ALU.mult,
                        accum_out=dots[0:1, g:g + 1])

            nc.vector.tensor_scalar(out_sb[:], dots[:], ct["clsb"][:], None,
                                    op0=ALU.add)
            nc.sync.dma_start(out_d[:], out_sb[:])

    nc.compile()
    return nc


def _prep_host(inputs):
    """Build the compact per-core input maps (vectorized, ~5 ms)."""
    f32 = np.float32
    # x [B, L, N] -> [B, 4, L*GC] chunk-major, zero-padded timestep each side
    x = np.asarray(inputs["x"], f32)
    xt = np.ascontiguousarray(
        x.reshape(B, L, 4, GC).transpose(0, 2, 1, 3)).reshape(B, 4, LGC)
    xg = np.zeros((B, 4, TC), _BF16)
    xg[:, :, GC:GC + LGC] = xt

    wp = np.zeros((128, WC), f32)
    w1 = np.asarray(inputs["conv1_w"], f32)
    for j in range(4):
        for k in range(3):
            wp[32 * j + k, P_WC1:P_WC1 + 32] = w1[:, 0, k]
    w2 = np.asarray(inputs["conv2_w"], f32)
    w3 = np.asarray(inputs["conv3_w"], f32)
    w4 = np.asarray(inputs["conv4_w"], f32)
    for k in range(3):
        wp[32 * k:32 * k + 32, P_WC2:P_WC2 + 64] = w2[:, :, k].T
    wp[0:64, P_WC3A:P_WC3A + 128] = w3[:, :, 0].T
    wp[64:128, P_WC3A:P_WC3A + 128] = w3[:, :, 1].T
    wp[0:64, P_WC3B:P_WC3B + 128] = w3[:, :, 2].T
    for k in range(3):
        for m in range(2):
            o = P_WC4 + 128 * (2 * k + m)
            wp[:, o:o + 128] = w4[128 * m:128 * m + 128, :, k].T
    fcw = np.asarray(inputs["fc_w"], f32)
    wp[:, P_FCW:P_FCW + 256] = fcw[:, 0:128].T / L
    wp[:, P_FCW + 256:P_FCW + 512] = fcw[:, 128:256].T / L
    for l in range(3):
        W = np.asarray(inputs[f"gat{l+1}_w"], f32)      # [256 out, 256 in]
        As = np.asarray(inputs[f"gat{l+1}_as"], f32)[0]  # [4, 64]
        Ad = np.asarray(inputs[f"gat{l+1}_ad"], f32)[0]
        for t in range(2):
            o = P_WTT + 256 * (2 * l + t)
            wp[:, o:o + 256] = W[:, 128 * t:128 * t + 128].T
        was = np.zeros((256, 8), f32)
        for hh in range(4):
            was[:, hh] = W[64 * hh:64 * hh + 64, :].T @ As[hh]
            was[:, 4 + hh] = W[64 * hh:64 * hh + 64, :].T @ Ad[hh]
        wp[:, P_WAS + 16 * l:P_WAS + 16 * l + 8] = was[0:128]
        wp[:, P_WAS + 16 * l + 8:P_WAS + 16 * l + 16] = was[128:256]
    ei = np.asarray(inputs["edge_index"])
    cnt = np.zeros((N, N), f32)
    np.add.at(cnt, (ei[1], ei[0]), 1.0)
    cnt += np.eye(N, dtype=f32)
    wp[:, P_CNT:P_CNT + 128] = cnt.T
    wp[:, P_ID:P_ID + 128] = np.eye(128, dtype=f32)
    wp = wp.astype(_BF16)

    pc_ = np.zeros((128, 8), f32)
    pc_[:, 0] = np.tile(np.asarray(inputs["conv1_b"], f32), 4)
    pc_[:, 1] = np.tile(np.asarray(inputs["conv2_b"], f32), 2)
    pc_[:, 2] = np.asarray(inputs["conv3_b"], f32)
    b4 = np.asarray(inputs["conv4_b"], f32)
    pc_[:, 3] = b4[0:128]
    pc_[:, 4] = b4[128:256]

    pr = np.zeros((1, RC), f32)
    pr[0, R_FCB:R_FCB + 256] = np.asarray(inputs["fc_b"], f32)
    for l in range(3):
        pr[0, R_GATB + 256 * l:R_GATB + 256 * l + 256] = \
            np.asarray(inputs[f"gat{l+1}_b"], f32)
        pr[0, R_NB + 256 * l:R_NB + 256 * l + 256] = \
            np.asarray(inputs[f"norm{l+1}_b"], f32)
        pr[0, R_MS + 256 * l:R_MS + 256 * l + 256] = \
            np.asarray(inputs[f"norm{l+1}_ms"], f32)
        pr[0, R_G + 256 * l:R_G + 256 * l + 256] = \
            np.asarray(inputs[f"norm{l+1}_g"], f32)
    pr[0, R_CLSW:R_CLSW + 256] = np.asarray(inputs["cls_w"], f32).reshape(-1)
    pr[0, R_CLSB] = np.asarray(inputs["cls_b"], f32).reshape(-1)[0]

    in_maps = []
    for core in range(NCORES):
        m = {"xprep": xg[core * GPC:(core + 1) * GPC],
             "packc": pc_, "packr": pr}
        if ALLGATHER:
            m["wsh"] = wp[16 * core:16 * core + 16]
        else:
            m["wpack"] = wp
        in_maps.append(m)
    return in_maps


def kernel(**inputs):
    from concourse.bass_utils import run_bass_kernel_spmd

    if "nc" not in _cache:
        _enable_jax_compile_cache()
        _cache["nc"] = _build_program()
    nc = _cache["nc"]

    in_maps = _prep_host(inputs)
    res = run_bass_kernel_spmd(nc, in_maps, list(range(NCORES)))
    out = np.zeros((B, 1), np.float32)
    for core in range(NCORES):
        o = np.asarray(res.results[core]["out"]).reshape(GPC)
        for g in range(GPC):
            out[core * GPC + g, 0] = o[g]
    return out
